# revision 14
# baseline (speedup 1.0000x reference)
"""AdditiveResonanceAttention kernel for 8x Trainium2 NeuronCores.

Sharding: 8 cores = (B=4) x (L/2). Core c handles batch b=c//2, query rows
[r0, r0+1024) with r0 = (c%2)*1024, all H=8 heads. Fully independent cores
(K/V projections are duplicated across the pair) -> no collectives.

Math notes:
- physics bias gamma*m_i*m_j*cos(phi_i-phi_j) is rank-2:
  = (g*m_i*cos phi_i)(m_j cos phi_j) + (g*m_i*sin phi_i)(m_j sin phi_j)
  -> folded into the QK^T matmul as 2 extra contraction rows (K=66).
- scores are computed TRANSPOSED (ST[j,i]) so that softmax's key-reduction
  is a ones-row fused into the P@V matmul, and P@V needs no transpose.
- softmax skips max-subtraction (scores are O(+-8); exp is exact in fp32).
- bv is folded on the host: softmax rows sum to 1 so v+=bv shifts ctx by bv
  exactly -> out += bv@Wo.T, merged with bo into the residual.
- the reference's fully-masked-row fallback (row_max <= -1e8) is not
  implemented; it cannot trigger for masks > -1e8 (here: zeros).
"""
import sys

for _p in ("/opt/trn_rl_repo", "/root/.axon_site/_ro/trn_rl_repo"):
    if _p not in sys.path:
        sys.path.insert(0, _p)

import numpy as np

import concourse.bass as bass
import concourse.bacc as bacc
import concourse.tile as tile
from concourse import mybir
from concourse.bass_utils import run_bass_kernel_spmd

F32 = mybir.dt.float32
F32R = mybir.dt.float32r
BF16 = mybir.dt.bfloat16
import os as _os
MM_DT = F32R if _os.environ.get("KERNEL_MM_DT", "bf16") == "f32r" else BF16
AF = mybir.ActivationFunctionType

B, L, D, H, DH = 4, 2048, 512, 8, 64
LQ = L // 2            # query rows per core
NCORES = 8
KEXT = DH + 2          # qk contraction with 2 physics rows
LN_EPS = 1e-12
NJT = L // 128         # 16 key tiles
NQB = LQ // 512        # 2 query blocks of 512
NDC = D // 128         # 4 feature chunks
HALF_PI = float(np.pi / 2.0)

_CACHED_NC = None


def build_nc():
    nc = bacc.Bacc()

    hT = nc.dram_tensor("hT", [D, L], MM_DT, kind="ExternalInput")
    hres = nc.dram_tensor("hres", [LQ, D], F32, kind="ExternalInput")
    wqT = nc.dram_tensor("wqT", [D, D], MM_DT, kind="ExternalInput")
    wkT = nc.dram_tensor("wkT", [D, D], MM_DT, kind="ExternalInput")
    wvT = nc.dram_tensor("wvT", [D, D], MM_DT, kind="ExternalInput")
    woT = nc.dram_tensor("woT", [D, D], MM_DT, kind="ExternalInput")
    phim = nc.dram_tensor("phim", [128, 128], F32, kind="ExternalInput")
    magm = nc.dram_tensor("magm", [128, 128], F32, kind="ExternalInput")
    maskc = nc.dram_tensor("maskc", [128, NJT], F32, kind="ExternalInput")
    bqc = nc.dram_tensor("bqc", [128, NDC], F32, kind="ExternalInput")
    bkc = nc.dram_tensor("bkc", [128, NDC], F32, kind="ExternalInput")
    gvec = nc.dram_tensor("gvec", [128, 1], F32, kind="ExternalInput")
    out = nc.dram_tensor("out", [LQ, D], F32, kind="ExternalOutput")

    with tile.TileContext(nc) as tc:
        _emit(nc, tc, locals())
    nc.compile()
    return nc


def _emit(nc, tc, t):
    from contextlib import ExitStack

    ts_ = bass.ts

    with ExitStack() as top:
        const = top.enter_context(tc.tile_pool(name="const", bufs=1))
        persist = top.enter_context(tc.tile_pool(name="persist", bufs=1))

        # ---- big input loads first: split across SP + ACT HWDGE queues ----
        hwp = top.enter_context(tc.tile_pool(name="hw", bufs=1))
        hT_sb = []
        for dc in range(NDC):
            tl = hwp.tile([128, L], MM_DT, tag=f"hT{dc}", name=f"hT{dc}")
            (nc.scalar if dc % 2 else nc.sync).dma_start(tl[:], t["hT"][ts_(dc, 128), :])
            hT_sb.append(tl)
        w_sb = {}
        for wi, nm in enumerate(("wkT", "wqT", "wvT")):
            w_sb[nm] = []
            for dc in range(NDC):
                tl = hwp.tile([128, D], MM_DT, tag=f"{nm}{dc}", name=f"{nm}{dc}")
                (nc.scalar if (wi * NDC + dc) % 2 else nc.sync).dma_start(
                    tl[:], t[nm][ts_(dc, 128), :])
                w_sb[nm].append(tl)

        # ---- constants ----
        mask_sb = const.tile([128, NJT], F32, tag="mask")
        nc.sync.dma_start(mask_sb[:], t["maskc"][:])
        bq_sb = const.tile([128, NDC], F32, tag="bq")
        nc.sync.dma_start(bq_sb[:], t["bqc"][:])
        bk_sb = const.tile([128, NDC], F32, tag="bk")
        nc.sync.dma_start(bk_sb[:], t["bkc"][:])
        g_sb = const.tile([128, 1], F32, tag="g")
        nc.sync.dma_start(g_sb[:], t["gvec"][:])
        ones8 = const.tile([128, H], F32, tag="ones8")
        nc.vector.memset(ones8[:], 1.0)
        halfpi = const.tile([128, 1], F32, tag="halfpi")
        nc.vector.memset(halfpi[:], HALF_PI)
        epst = const.tile([128, 1], F32, tag="epst")
        nc.vector.memset(epst[:], LN_EPS)

        # ---- persistent attention operands ----
        kT = [persist.tile([KEXT, L], MM_DT, tag=f"kT{h}", name=f"kT{h}") for h in range(H)]
        qT = [persist.tile([KEXT, LQ], MM_DT, tag=f"qT{h}", name=f"qT{h}") for h in range(H)]
        # v_sb[jt]: [128, H*(DH+1)] blocks of (64 v-cols + 1 ones-col) per head
        v_sb = [persist.tile([128, H * (DH + 1)], MM_DT, tag=f"v{jt}", name=f"v{jt}") for jt in range(NJT)]
        # ctxT pairs: heads (2c, 2c+1) stacked on partitions
        ctxT = [persist.tile([128, LQ], MM_DT, tag=f"ctxT{c}", name=f"ctxT{c}") for c in range(NDC)]

        # ---- physics: mc=mag*cos(phi), ms=mag*sin(phi), g* variants ----
        with tc.tile_pool(name="phys", bufs=1) as php:
            phi_sb = php.tile([128, 128], F32, tag="phi")
            nc.sync.dma_start(phi_sb[:], t["phim"][:])
            mag_sb = php.tile([128, 128], F32, tag="mag")
            nc.sync.dma_start(mag_sb[:], t["magm"][:])
            # ACT Sin is only valid on [-pi, pi]: range-reduce via
            # round-to-nearest int cast (n = rint(arg/2pi); arg -= 2pi*n).
            AO = mybir.AluOpType
            TWO_PI = 2.0 * float(np.pi)

            def _reduce(nm, pre_add):
                tt = php.tile([128, 128], F32, tag=f"t{nm}", name=f"t{nm}")
                nc.vector.tensor_scalar(tt[:], phi_sb[:], 1.0 / TWO_PI,
                                        pre_add / TWO_PI, AO.mult, AO.add)
                ti = php.tile([128, 128], mybir.dt.int32, tag=f"ti{nm}",
                              name=f"ti{nm}")
                nc.vector.tensor_copy(ti[:], tt[:])
                tf = php.tile([128, 128], F32, tag=f"tf{nm}", name=f"tf{nm}")
                nc.vector.tensor_copy(tf[:], ti[:])
                red = php.tile([128, 128], F32, tag=f"red{nm}", name=f"red{nm}")
                nc.vector.scalar_tensor_tensor(red[:], tf[:], -TWO_PI,
                                               phi_sb[:], AO.mult, AO.add)
                return red

            zero_t = php.tile([128, 1], F32, tag="zero")
            nc.vector.memset(zero_t[:], 0.0)
            cosm = php.tile([128, 128], F32, tag="cos")
            nc.scalar.activation(cosm[:], _reduce("c", HALF_PI)[:], AF.Sin,
                                 bias=halfpi[:, 0:1])
            sinm = php.tile([128, 128], F32, tag="sin")
            nc.scalar.activation(sinm[:], _reduce("s", 0.0)[:], AF.Sin,
                                 bias=zero_t[:, 0:1])
            mc = php.tile([128, 128], MM_DT, tag="mc")
            nc.vector.tensor_mul(mc[:], cosm[:], mag_sb[:])
            ms = php.tile([128, 128], MM_DT, tag="ms")
            nc.vector.tensor_mul(ms[:], sinm[:], mag_sb[:])
            gmc = php.tile([128, 128], MM_DT, tag="gmc")
            nc.vector.tensor_scalar_mul(gmc[:], mc[:], g_sb[:, 0:1])
            gms = php.tile([128, 128], MM_DT, tag="gms")
            nc.vector.tensor_scalar_mul(gms[:], ms[:], g_sb[:, 0:1])
            # distribute: phys rows 64/65 of kT (full L) and qT (first LQ of
            # the host-permuted key order = this core's query rows)
            for h in range(H):
                nc.gpsimd.dma_start(kT[h][64:65, :], mc[h * 16:(h + 1) * 16, :])
                nc.gpsimd.dma_start(kT[h][65:66, :], ms[h * 16:(h + 1) * 16, :])
                nc.gpsimd.dma_start(qT[h][64:65, :], gmc[h * 16:h * 16 + 8, :])
                nc.gpsimd.dma_start(qT[h][65:66, :], gms[h * 16:h * 16 + 8, :])

        # ---- projections ----
        with tc.tile_pool(name="ptmp", bufs=6) as ptmp, \
             tc.tile_pool(name="projp", bufs=4, space="PSUM") as projp:
            # k projection: kT[dout, l] for all l
            for oc in range(NDC):
                for lb in range(L // 512):
                    ps = projp.tile([128, 512], F32, tag="proj")
                    for dc in range(NDC):
                        nc.tensor.matmul(
                            ps[:], w_sb["wkT"][dc][:, ts_(oc, 128)],
                            hT_sb[dc][:, ts_(lb, 512)],
                            start=(dc == 0), stop=(dc == NDC - 1))
                    pair = ptmp.tile([128, 512], MM_DT, tag="pair")
                    nc.vector.tensor_scalar_add(pair[:], ps[:], bk_sb[:, oc:oc + 1])
                    nc.gpsimd.dma_start(kT[2 * oc][0:64, ts_(lb, 512)], pair[0:64, :])
                    nc.gpsimd.dma_start(kT[2 * oc + 1][0:64, ts_(lb, 512)], pair[64:128, :])

            # q projection: only this core's LQ rows, pre-scaled by 1/8
            for oc in range(NDC):
                for qb in range(NQB):
                    ps = projp.tile([128, 512], F32, tag="proj")
                    for dc in range(NDC):
                        nc.tensor.matmul(
                            ps[:], w_sb["wqT"][dc][:, ts_(oc, 128)],
                            hT_sb[dc][:, ts_(qb, 512)],
                            start=(dc == 0), stop=(dc == NDC - 1))
                    pair = ptmp.tile([128, 512], MM_DT, tag="pair")
                    nc.vector.tensor_scalar(
                        pair[:], ps[:], 0.125, bq_sb[:, oc:oc + 1],
                        mybir.AluOpType.mult, mybir.AluOpType.add)
                    nc.gpsimd.dma_start(qT[2 * oc][0:64, ts_(qb, 512)], pair[0:64, :])
                    nc.gpsimd.dma_start(qT[2 * oc + 1][0:64, ts_(qb, 512)], pair[64:128, :])

            # v projection: natural layout [l, dout] into strided v_sb + ones col
            for jt in range(NJT):
                ps = projp.tile([128, 512], F32, tag="proj")
                for dc in range(NDC):
                    nc.tensor.matmul(
                        ps[:], hT_sb[dc][:, ts_(jt, 128)],
                        w_sb["wvT"][dc][:, 0:D],
                        start=(dc == 0), stop=(dc == NDC - 1))
                vv = v_sb[jt][:].rearrange("p (h d) -> p h d", h=H)
                nc.vector.tensor_copy(
                    vv[:, :, 0:DH], ps[:].rearrange("p (h d) -> p h d", h=H))
                nc.vector.tensor_copy(vv[:, :, DH:DH + 1], ones8[:, :, None])

        # ---- attention ----
        with tc.tile_pool(name="stp", bufs=2, space="PSUM") as stp, \
             tc.tile_pool(name="pvp", bufs=4, space="PSUM") as pvp, \
             tc.tile_pool(name="epool", bufs=4) as epool, \
             tc.tile_pool(name="dnp", bufs=2) as dnp, \
             tc.tile_pool(name="bcp", bufs=3) as bcp:
            for h in range(H):
                pv = [pvp.tile([65, 512], F32, tag="pv", name=f"pv{h}_{i}") for i in range(NQB)]
                for jt in range(NJT):
                    st = stp.tile([128, LQ], F32, tag="st")
                    for qb in range(NQB):
                        nc.tensor.matmul(
                            st[:, ts_(qb, 512)],
                            kT[h][:, ts_(jt, 128)], qT[h][:, ts_(qb, 512)],
                            start=True, stop=True)
                    e = epool.tile([128, LQ], MM_DT, tag="e")
                    nc.scalar.activation(e[:], st[:], AF.Exp,
                                         bias=mask_sb[:, jt:jt + 1])
                    for qb in range(NQB):
                        nc.tensor.matmul(
                            pv[qb][:], v_sb[jt][:, h * 65:(h + 1) * 65],
                            e[:, ts_(qb, 512)],
                            start=(jt == 0), stop=(jt == NJT - 1))
                # denominators: stage row 64, gather, recip, scatter, bcast
                stg = dnp.tile([65, LQ], F32, tag="stg")
                for qb in range(NQB):
                    nc.vector.tensor_copy(stg[64:65, ts_(qb, 512)], pv[qb][64:65, :])
                dn = dnp.tile([16, 64], F32, tag="dn")
                nc.sync.dma_start(dn[:], stg[64:65, :])
                di = dnp.tile([16, 64], F32, tag="di")
                nc.vector.reciprocal(di[:], dn[:])
                for qb in range(NQB):
                    drow = dnp.tile([1, 512], F32, tag="drow")
                    nc.sync.dma_start(drow[0:1, :], di[qb * 8:(qb + 1) * 8, :])
                    bc = bcp.tile([64, 512], F32, tag="bc")
                    nc.gpsimd.partition_broadcast(bc[:], drow[0:1, :])
                    if h % 2 == 0:
                        nc.vector.tensor_mul(
                            ctxT[h // 2][0:64, ts_(qb, 512)], pv[qb][0:64, :], bc[:])
                    else:
                        tmp = bcp.tile([64, 512], MM_DT, tag="ctmp")
                        nc.vector.tensor_mul(tmp[:], pv[qb][0:64, :], bc[:])
                        nc.gpsimd.dma_start(ctxT[h // 2][64:128, ts_(qb, 512)], tmp[:])

        # ---- output projection + residual + layernorm ----
        with tc.tile_pool(name="wo", bufs=1) as wop, \
             tc.tile_pool(name="resp", bufs=4) as resp, \
             tc.tile_pool(name="lnp", bufs=4) as lnp, \
             tc.tile_pool(name="outp", bufs=2, space="PSUM") as outp:
            wo_sb = []
            for dc in range(NDC):
                tl = wop.tile([128, D], MM_DT, tag=f"wo{dc}")
                nc.scalar.dma_start(tl[:], t["woT"][ts_(dc, 128), :])
                wo_sb.append(tl)
            for lc in range(LQ // 128):
                ps = outp.tile([128, D], F32, tag="o")
                for dc in range(NDC):
                    nc.tensor.matmul(
                        ps[:], ctxT[dc][:, ts_(lc, 128)], wo_sb[dc][:, 0:D],
                        start=(dc == 0), stop=(dc == NDC - 1))
                res = resp.tile([128, D], F32, tag="res")
                nc.scalar.dma_start(res[:], t["hres"][ts_(lc, 128), :])
                x = lnp.tile([128, D], F32, tag="x")
                nc.vector.tensor_add(x[:], ps[:], res[:])
                stats = lnp.tile([128, 6], F32, tag="stats")
                nc.vector.bn_stats(stats[:], x[:])
                mv = lnp.tile([128, 2], F32, tag="mv")
                nc.vector.bn_aggr(mv[:], stats[:])
                sd = lnp.tile([128, 1], F32, tag="sd")
                nc.scalar.activation(sd[:], mv[:, 1:2], AF.Sqrt, bias=epst[:, 0:1])
                rstd = lnp.tile([128, 1], F32, tag="rstd")
                nc.vector.reciprocal(rstd[:], sd[:])
                nmr = lnp.tile([128, 1], F32, tag="nmr")
                nc.vector.scalar_tensor_tensor(
                    nmr[:], mv[:, 0:1], -1.0, rstd[:, 0:1],
                    mybir.AluOpType.mult, mybir.AluOpType.mult)
                o = lnp.tile([128, D], F32, tag="oo")
                nc.scalar.activation(o[:], x[:], AF.Identity,
                                     bias=nmr[:, 0:1], scale=rstd[:, 0:1])
                nc.scalar.dma_start(t["out"][ts_(lc, 128), :], o[:])


# SPMD: the same program runs on all 8 cores, so the per-core query-half
# offset is rolled into the HOST input layout instead: every per-key input
# (hT columns, phi, mag, mask) is cyclically permuted so the core's query
# rows come first. Softmax attention is exactly invariant under a key
# permutation applied consistently to k/v/phi/mag/mask.


def _host_prep(inputs):
    hs = np.ascontiguousarray(np.asarray(inputs["hidden_states"], dtype=np.float32))
    am = np.asarray(inputs["attention_mask"], dtype=np.float32)
    phi = np.asarray(inputs["phi"], dtype=np.float32)
    mag = np.asarray(inputs["mag"], dtype=np.float32)
    Wq = np.asarray(inputs["Wq"], dtype=np.float32)
    Wk = np.asarray(inputs["Wk"], dtype=np.float32)
    Wv = np.asarray(inputs["Wv"], dtype=np.float32)
    Wo = np.asarray(inputs["Wo"], dtype=np.float32)
    bq = np.asarray(inputs["bq"], dtype=np.float32)
    bk = np.asarray(inputs["bk"], dtype=np.float32)
    bv = np.asarray(inputs["bv"], dtype=np.float32)
    bo = np.asarray(inputs["bo"], dtype=np.float32)
    gamma = np.asarray(inputs["gamma"], dtype=np.float32).reshape(H)
    ln_w = np.asarray(inputs["ln_w"], dtype=np.float32)
    ln_b = np.asarray(inputs["ln_b"], dtype=np.float32)
    assert np.allclose(ln_w, 1.0) and np.allclose(ln_b, 0.0), \
        "kernel folds ln affine away; generalize if this fires"

    bo_eff = bo + bv @ Wo.T
    np_mm = mybir.dt.np(MM_DT)
    wqT = np.ascontiguousarray(Wq.T).astype(np_mm)
    wkT = np.ascontiguousarray(Wk.T).astype(np_mm)
    wvT = np.ascontiguousarray(Wv.T).astype(np_mm)
    woT = np.ascontiguousarray(Wo.T).astype(np_mm)
    gvec = np.repeat(gamma, 16)[:, None].astype(np.float32)
    bqc = np.ascontiguousarray((bq / 8.0).reshape(NDC, 128).T)
    bkc = np.ascontiguousarray(bk.reshape(NDC, 128).T)

    in_maps = []
    for c in range(NCORES):
        b, half = c // 2, c % 2
        r0 = half * LQ
        perm = np.roll(np.arange(L), -r0)  # query half first
        hTb = np.ascontiguousarray(hs[b].T[:, perm]).astype(np_mm)
        phib = np.ascontiguousarray(phi[b][:, perm]).reshape(128, 128)
        magb = np.ascontiguousarray(mag[b][:, perm]).reshape(128, 128)
        maskb = np.ascontiguousarray(am[b, 0, 0][perm].reshape(NJT, 128).T)
        hresb = np.ascontiguousarray(hs[b, r0:r0 + LQ]) + bo_eff[None, :]
        in_maps.append(dict(
            hT=hTb, hres=hresb.astype(np.float32),
            wqT=wqT, wkT=wkT, wvT=wvT, woT=woT,
            phim=phib, magm=magb, maskc=maskb,
            bqc=bqc, bkc=bkc, gvec=gvec,
        ))
    return in_maps


def _get_nc():
    global _CACHED_NC
    if _CACHED_NC is None:
        _CACHED_NC = build_nc()
    return _CACHED_NC


def run(inputs, **spmd_kwargs):
    in_maps = _host_prep(inputs)
    nc = _get_nc()
    res = run_bass_kernel_spmd(nc, in_maps, core_ids=list(range(NCORES)),
                               **spmd_kwargs)
    out = np.empty((B, L, D), dtype=np.float32)
    for c in range(NCORES):
        b, half = c // 2, c % 2
        out[b, half * LQ:(half + 1) * LQ] = res.results[c]["out"]
    return out, res


def kernel(**inputs) -> np.ndarray:
    out, _ = run(inputs)
    return out
